# revision 1
# baseline (speedup 1.0000x reference)
"""Multi-head causal self-attention (B=4, T=1024, d_model=2048, 16 heads of 128)
for 8 Trainium2 NeuronCores.

Sharding: hybrid data x tensor parallel. Core c handles batch b = c//2 and
head group g = c%2 (8 heads per core). Each core computes q/k/v projections
for its 8 heads, causal flash-style attention, and the out-projection rows
for those heads, producing a partial [1024, 2048] output for its batch.
The host sums the two partials per batch and adds the output bias.

All on-device layouts are feature-major so no transposes are needed anywhere:
  - x is shipped pre-transposed per batch: xt [2048, 1024] (fp16)
  - q, k are produced feature-major [dh, T] per head; v token-major [T, dh]
  - scores are computed transposed: S^T[kv, q] = k_fm.T @ q_fm (lhsT=k, rhs=q)
  - softmax denominator via ones[128,128] matmul (partition reduction on PE),
    which also broadcasts the per-q sum to all 128 partitions
  - attention output accumulates as out^T[dh, q] = v_tm.T @ exp(S^T)
  - out^T is exactly the lhsT the out-projection needs

Heads are processed in two blocks of 4 so projection weights and q/k/v
activations fit in SBUF alongside the resident x^T and w_out. Within a
block, attention is computed for two heads interleaved so PE matmuls hide
the ACT exp latency. Inputs are DMA'd in per-k-chunk tiles so the first
projection matmuls start ~2us in instead of waiting for monolithic loads.
"""

import numpy as np

B, T, C = 4, 1024, 2048
H = 16          # total heads
HL = 8          # heads per core (local)
HB = 4          # heads per block
DH = 128        # head dim
KC = C // 128   # contraction chunks (16)
P = 128
NCORES = 8

_cache = {}


def _build():
    import concourse.bacc as bacc
    import concourse.mybir as mybir
    import concourse.tile as tile

    F32 = mybir.dt.float32
    F16 = mybir.dt.float16
    AF = mybir.ActivationFunctionType
    ALU = mybir.AluOpType

    nc = bacc.Bacc("TRN2", target_bir_lowering=False, debug=False)

    xt_d = nc.dram_tensor("xt", (C, T), F16, kind="ExternalInput")
    wq_d = nc.dram_tensor("wq", (C, HL * DH), F16, kind="ExternalInput")
    wk_d = nc.dram_tensor("wk", (C, HL * DH), F16, kind="ExternalInput")
    wv_d = nc.dram_tensor("wv", (C, HL * DH), F16, kind="ExternalInput")
    wo_d = nc.dram_tensor("wo", (HL * DH, C), F16, kind="ExternalInput")
    bq_d = nc.dram_tensor("bq", (P, HL), F32, kind="ExternalInput")
    bk_d = nc.dram_tensor("bk", (P, HL), F32, kind="ExternalInput")
    bvb_d = nc.dram_tensor("bvb", (P, HL * DH), F32, kind="ExternalInput")
    mask_d = nc.dram_tensor("mask", (P, P), F32, kind="ExternalInput")
    part_d = nc.dram_tensor("part", (T, C), F32, kind="ExternalOutput")

    BW = HB * DH  # head-block feature width (512)

    xt_v = xt_d.rearrange("(o p) t -> p o t", p=P)
    wq_v = wq_d.rearrange("(o p) m -> p o m", p=P)
    wk_v = wk_d.rearrange("(o p) m -> p o m", p=P)
    wv_v = wv_d.rearrange("(o p) m -> p o m", p=P)

    with tile.TileContext(nc) as tc:
        with (
            tc.tile_pool(name="res", bufs=1) as res,
            tc.tile_pool(name="wblk", bufs=1) as wblk,
            tc.tile_pool(name="qkv", bufs=2) as qkv,
            tc.tile_pool(name="wp", bufs=3) as wp,
            tc.tile_pool(name="ps", bufs=3, space="PSUM") as ps,
        ):
            bq_sb = res.tile([P, HL], F32, tag="bq")
            bk_sb = res.tile([P, HL], F32, tag="bk")
            bvb_sb = res.tile([P, HL * DH], F32, tag="bvb")
            mask_sb = res.tile([P, P], F32, tag="mask")

            ones_sb = res.tile([P, P], F16, tag="ones")
            nc.vector.memset(ones_sb[:], 1.0)

            # Warm the PE (HAM un-throttles after ~3.4us of activity) while the
            # input DMAs stream in; these matmuls depend only on the memset.
            warm = ps.tile([P, P], F32, tag="mm")
            for _ in range(48):
                nc.tensor.matmul(warm[:], ones_sb[:], ones_sb[:], start=True, stop=True)

            # x^T in per-k-chunk tiles so compute starts after the first chunks
            xts = []
            for kc in range(KC):
                xt_sb = res.tile([P, T], F16, tag=f"xt{kc}", name=f"xt{kc}")
                xts.append(xt_sb)
            wts = {w: [None] * KC for w in ("wq", "wk", "wv")}

            def dma_block_weights(blk):
                lo = blk * BW

                def load_w(wname, wv_, kc):
                    wt = wblk.tile(
                        [P, BW], F16, tag=f"{wname}{kc}", name=f"{wname}{kc}_{blk}"
                    )
                    nc.sync.dma_start(wt[:], wv_[:, kc, lo : lo + BW])
                    wts[wname][kc] = wt

                if blk == 0:
                    # arrival order matches first consumption: the h=0 q-proj
                    # k-chain needs (xt[kc], wq[kc]) pairs in kc order
                    for kc in range(KC):
                        nc.sync.dma_start(xts[kc][:], xt_v[:, kc, :])
                        load_w("wq", wq_v, kc)
                    nc.sync.dma_start(bq_sb[:], bq_d[:])
                    nc.sync.dma_start(bk_sb[:], bk_d[:])
                    nc.sync.dma_start(bvb_sb[:], bvb_d[:])
                    nc.sync.dma_start(mask_sb[:], mask_d[:])
                    for kc in range(KC):
                        load_w("wk", wk_v, kc)
                    for kc in range(KC):
                        load_w("wv", wv_v, kc)
                else:
                    for kc in range(KC):
                        load_w("wq", wq_v, kc)
                        load_w("wk", wk_v, kc)
                        load_w("wv", wv_v, kc)

            wo_sb = res.tile([P, HL, C], F16, tag="wo")
            oT = res.tile([P, HL, T], F16, tag="oT")

            for blk in range(HL // HB):
                lo = blk * BW
                dma_block_weights(blk)

                qf = qkv.tile([P, HB, T], F16, tag="qf")
                kf = qkv.tile([P, HB, T], F16, tag="kf")
                vt = qkv.tile([P, T // P, BW], F16, tag="vt")

                # ---- Phase 1: projections for this block ----
                for h in range(HB):
                    for dst, wname, bsb in (("qf", "wq", bq_sb), ("kf", "wk", bk_sb)):
                        dtile = qf if dst == "qf" else kf
                        for t in range(T // 512):
                            pt = ps.tile([P, 512], F32, tag="mm")
                            for kc in range(KC):
                                nc.tensor.matmul(
                                    pt[:],
                                    wts[wname][kc][:, h * DH : (h + 1) * DH],
                                    xts[kc][:, t * 512 : (t + 1) * 512],
                                    start=(kc == 0),
                                    stop=(kc == KC - 1),
                                )
                            nc.vector.tensor_tensor(
                                dtile[:, h, t * 512 : (t + 1) * 512],
                                pt[:],
                                bsb[
                                    :, blk * HB + h : blk * HB + h + 1
                                ].to_broadcast((P, 512)),
                                ALU.add,
                            )
                for m in range(T // P):
                    pt = ps.tile([P, 512], F32, tag="mm")
                    for kc in range(KC):
                        nc.tensor.matmul(
                            pt[:],
                            xts[kc][:, m * P : (m + 1) * P],
                            wts["wv"][kc][:],
                            start=(kc == 0),
                            stop=(kc == KC - 1),
                        )
                    nc.vector.tensor_tensor(
                        vt[:, m, :], pt[:], bvb_sb[:, lo : lo + BW], ALU.add
                    )

                if blk == 0:
                    # out-proj weights: needed only in phase 3; load mid-kernel
                    nc.sync.dma_start(
                        wo_sb[:], wo_d.rearrange("(h p) n -> p h n", p=P)
                    )

                # ---- Phase 2: causal attention, two heads interleaved ----
                for hp in range(HB // 2):
                    pair = (2 * hp, 2 * hp + 1)  # local head idx within block
                    for qc in range(T // 512):
                        jmax = (qc + 1) * 4
                        att = {}
                        den = {}
                        for l in pair:
                            att[l] = ps.tile(
                                [P, 512], F32, tag="att", bufs=3, name=f"att{l}"
                            )
                            den[l] = ps.tile(
                                [P, 512], F32, tag="den", bufs=2, name=f"den{l}"
                            )

                        def bounds(j):
                            s = max(512 * qc, 128 * j)
                            return s, 512 * qc + 512 - s

                        sts = {}

                        def issue_st(l, j):
                            s, n = bounds(j)
                            st = ps.tile([P, 512], F32, tag="mm", name=f"st{l}")
                            nc.tensor.matmul(
                                st[:, :n],
                                kf[:, l, j * P : (j + 1) * P],
                                qf[:, l, s : 512 * qc + 512],
                                start=True,
                                stop=True,
                            )
                            if 128 * j >= 512 * qc:
                                nc.vector.tensor_tensor(
                                    st[:, :P], st[:, :P], mask_sb[:], ALU.add
                                )
                            sts[(l, j)] = st

                        for l in pair:
                            issue_st(l, 0)
                        for j in range(jmax):
                            s, n = bounds(j)
                            c0 = s - 512 * qc
                            for l in pair:
                                st = sts.pop((l, j))
                                E = wp.tile([P, 512], F16, tag="E", bufs=6)
                                nc.scalar.activation(E[:, :n], st[:, :n], AF.Exp)
                                if j + 1 < jmax:
                                    issue_st(l, j + 1)
                                nc.tensor.matmul(
                                    att[l][:, c0:],
                                    vt[:, j, l * DH : (l + 1) * DH],
                                    E[:, :n],
                                    start=(j == 0),
                                    stop=(j == jmax - 1),
                                )
                                nc.tensor.matmul(
                                    den[l][:, c0:],
                                    ones_sb[:],
                                    E[:, :n],
                                    start=(j == 0),
                                    stop=(j == jmax - 1),
                                )
                        for l in pair:
                            hh = blk * HB + l
                            rc = wp.tile([P, 512], F32, tag="rc")
                            nc.vector.reciprocal_approx_fast(rc[:], den[l][:])
                            nc.vector.tensor_tensor(
                                oT[:, hh, qc * 512 : (qc + 1) * 512],
                                att[l][:],
                                rc[:],
                                ALU.mult,
                            )

            # ---- Phase 3: out projection (partial over this core's heads) ----
            part_v = part_d.rearrange("(mo p) n -> p mo n", p=P)
            for m in range(T // P):
                for n2 in range(C // 512):
                    pt = ps.tile([P, 512], F32, tag="mm")
                    for h in range(HL):
                        nc.tensor.matmul(
                            pt[:],
                            oT[:, h, m * P : (m + 1) * P],
                            wo_sb[:, h, n2 * 512 : (n2 + 1) * 512],
                            start=(h == 0),
                            stop=(h == HL - 1),
                        )
                    po = wp.tile([P, 512], F32, tag="po")
                    nc.vector.tensor_copy(po[:], pt[:])
                    nc.sync.dma_start(part_v[:, m, n2 * 512 : (n2 + 1) * 512], po[:])

    nc.compile()
    return nc


def _prep_inputs(x, w_qkv, b_qkv, w_out):
    """Build the 8 per-core input maps (host-side shard + layout prep)."""
    f16 = np.float16
    scale = np.float32(1.0 / np.sqrt(DH))

    xt = [np.ascontiguousarray(x[b].T).astype(f16) for b in range(B)]

    mask = np.where(
        np.arange(P)[None, :] >= np.arange(P)[:, None], 0.0, -1e30
    ).astype(np.float32)

    per_g = []
    for g in range(2):
        lo, hi = g * HL * DH, (g + 1) * HL * DH
        wq = np.ascontiguousarray(w_qkv[:, lo:hi] * scale).astype(f16)
        wk = np.ascontiguousarray(w_qkv[:, C + lo : C + hi]).astype(f16)
        wv = np.ascontiguousarray(w_qkv[:, 2 * C + lo : 2 * C + hi]).astype(f16)
        wo = np.ascontiguousarray(w_out[lo:hi, :]).astype(f16)
        bq = (b_qkv[lo:hi] * scale).astype(np.float32).reshape(HL, P).T.copy()
        bk = b_qkv[C + lo : C + hi].astype(np.float32).reshape(HL, P).T.copy()
        bv = b_qkv[2 * C + lo : 2 * C + hi].astype(np.float32)
        bvb = np.ascontiguousarray(np.broadcast_to(bv[None, :], (P, HL * DH)))
        per_g.append(dict(wq=wq, wk=wk, wv=wv, wo=wo, bq=bq, bk=bk, bvb=bvb))

    in_maps = []
    for c in range(NCORES):
        b, g = c // 2, c % 2
        m = dict(per_g[g])
        m["xt"] = xt[b]
        m["mask"] = mask
        in_maps.append(m)
    return in_maps


def run(x, w_qkv, b_qkv, w_out, b_out, trace=False, **trace_kwargs):
    from concourse.bass_utils import run_bass_kernel_spmd

    x = np.asarray(x, dtype=np.float32)
    w_qkv = np.asarray(w_qkv, dtype=np.float32)
    b_qkv = np.asarray(b_qkv, dtype=np.float32)
    w_out = np.asarray(w_out, dtype=np.float32)
    b_out = np.asarray(b_out, dtype=np.float32)

    if "nc" not in _cache:
        _cache["nc"] = _build()
    nc = _cache["nc"]

    in_maps = _prep_inputs(x, w_qkv, b_qkv, w_out)
    res = run_bass_kernel_spmd(
        nc, in_maps, core_ids=list(range(NCORES)), trace=trace, **trace_kwargs
    )

    out = np.empty((B, T, C), np.float32)
    for b in range(B):
        out[b] = res.results[2 * b]["part"] + res.results[2 * b + 1]["part"]
    out += b_out
    return out, res


def kernel(x, w_qkv, b_qkv, w_out, b_out):
    out, _ = run(x, w_qkv, b_qkv, w_out, b_out)
    return out



# revision 2
# speedup vs baseline: 1.2205x; 1.2205x over previous
"""Multi-head causal self-attention (B=4, T=1024, d_model=2048, 16 heads of 128)
for 8 Trainium2 NeuronCores.

Sharding: hybrid data x tensor parallel. Core c handles batch b = c//2 and
head group g = c%2 (8 heads per core). Each core computes q/k/v projections
for its 8 heads, causal flash-style attention, and the out-projection rows
for those heads, producing a partial [1024, 2048] output for its batch.
The host sums the two partials per batch and adds the output bias.

Precision: q/k projections run in fp8(e4m3) with DoubleRow matmuls (2 k-tiles
of 128 contracted per pass -> 2x PE throughput); softmax makes the resulting
~5% q/k noise nearly invisible in the output (rel err ~1e-2 vs the 2e-2 gate)
because attention-weight wiggle only re-mixes exact fp16 v rows. The v path,
scores, AV, denominator and out-projection stay fp16 (v and out-proj errors
pass straight through to the output, so fp8 there fails the gate). Weights
are pre-scaled by 64 before fp8 quantization (w ~ +-0.022 would be subnormal
in e4m3); the 1/64 and the 1/sqrt(dh) score scale are folded into the fused
scale+bias (scalar_tensor_tensor) that moves q/k from PSUM to SBUF.

All on-device layouts are feature-major so no transposes are needed anywhere:
  - x is shipped per batch as xt [2048, 1024] twice: fp16 (v path) and fp8
    in DoubleRow pair layout (q/k path)
  - q, k are produced feature-major [dh, T] per head; v token-major [T, dh]
  - scores are computed transposed: S^T[kv, q] = k_fm.T @ q_fm (lhsT=k, rhs=q)
  - softmax denominator via ones[128,128] matmul (partition reduction on PE),
    which also broadcasts the per-q sum to all 128 partitions
  - attention output accumulates as out^T[dh, q] = v_tm.T @ exp(S^T)
  - out^T is exactly the lhsT the out-projection needs

Heads are processed in two blocks of 4 so projection weights and q/k/v
activations fit in SBUF alongside the resident x^T and w_out. Within a
block, attention is computed for two heads interleaved so PE matmuls hide
the ACT exp latency. Inputs are DMA'd in per-k-chunk tiles so the first
projection matmuls start ~1us in instead of waiting for monolithic loads.
The output partial is written fp16 (halves the tail DMA); the host sums the
two partials per batch in fp32.
"""

import numpy as np
import ml_dtypes

B, T, C = 4, 1024, 2048
H = 16          # total heads
HL = 8          # heads per core (local)
HB = 4          # heads per block
DH = 128        # head dim
KC = C // 128   # contraction chunks (16)
KP = KC // 2    # DoubleRow chunk pairs (8)
P = 128
NCORES = 8
SW = 64.0       # fp8 weight pre-scale

_cache = {}


def _build():
    import concourse.bacc as bacc
    import concourse.mybir as mybir
    import concourse.tile as tile

    F32 = mybir.dt.float32
    F16 = mybir.dt.float16
    F8 = mybir.dt.float8e4
    AF = mybir.ActivationFunctionType
    ALU = mybir.AluOpType
    DR = mybir.MatmulPerfMode.DoubleRow

    rs = float(1.0 / np.sqrt(DH))

    nc = bacc.Bacc("TRN2", target_bir_lowering=False, debug=False)

    xt_d = nc.dram_tensor("xt", (C, T), F16, kind="ExternalInput")
    xt8_d = nc.dram_tensor("xt8", (C, T), F8, kind="ExternalInput")
    wq_d = nc.dram_tensor("wq", (C, HL * DH), F8, kind="ExternalInput")
    wk_d = nc.dram_tensor("wk", (C, HL * DH), F8, kind="ExternalInput")
    wv_d = nc.dram_tensor("wv", (C, HL * DH), F16, kind="ExternalInput")
    wo_d = nc.dram_tensor("wo", (HL * DH, C), F16, kind="ExternalInput")
    bq_d = nc.dram_tensor("bq", (P, HL), F32, kind="ExternalInput")
    bk_d = nc.dram_tensor("bk", (P, HL), F32, kind="ExternalInput")
    bvb_d = nc.dram_tensor("bvb", (P, HL * DH), F32, kind="ExternalInput")
    mask_d = nc.dram_tensor("mask", (P, P), F32, kind="ExternalInput")
    part_d = nc.dram_tensor("part", (T, C), F16, kind="ExternalOutput")

    BW = HB * DH  # head-block feature width (512)

    xt_v = xt_d.rearrange("(o p) t -> p o t", p=P)
    xt8_v = xt8_d.rearrange("(o two p) t -> p o two t", p=P, two=2)
    wq_v = wq_d.rearrange("(o two p) m -> p o two m", p=P, two=2)
    wk_v = wk_d.rearrange("(o two p) m -> p o two m", p=P, two=2)
    wv_v = wv_d.rearrange("(o p) m -> p o m", p=P)

    with tile.TileContext(nc) as tc:
        with (
            tc.tile_pool(name="res", bufs=1) as res,
            tc.tile_pool(name="wblk", bufs=1) as wblk,
            tc.tile_pool(name="qkv", bufs=2) as qkv,
            tc.tile_pool(name="wp", bufs=3) as wp,
            tc.tile_pool(name="ps", bufs=3, space="PSUM") as ps,
        ):
            bq_sb = res.tile([P, HL], F32, tag="bq")
            bk_sb = res.tile([P, HL], F32, tag="bk")
            bvb_sb = res.tile([P, HL * DH], F32, tag="bvb")
            mask_sb = res.tile([P, P], F32, tag="mask")

            ones_sb = res.tile([P, P], F16, tag="ones")
            nc.vector.memset(ones_sb[:], 1.0)

            # Warm the PE (HAM un-throttles after ~3.4us of activity) while the
            # input DMAs stream in; these matmuls depend only on the memset.
            warm = ps.tile([P, P], F32, tag="mm")
            for _ in range(48):
                nc.tensor.matmul(warm[:], ones_sb[:], ones_sb[:], start=True, stop=True)

            # x^T fp16 per-k-chunk (v path) and fp8 per-pair (q/k path)
            xts = []
            for kc in range(KC):
                xts.append(res.tile([P, T], F16, tag=f"xt{kc}", name=f"xt{kc}"))
            xt8s = []
            for o in range(KP):
                xt8s.append(res.tile([P, 2, T], F8, tag=f"xt8_{o}", name=f"xt8_{o}"))
            wts = {"wv": [None] * KC}
            w8s = {"wq": [None] * KP, "wk": [None] * KP}

            def dma_block_weights(blk):
                lo = blk * BW

                def load_w8(wname, wv_, o):
                    wt = wblk.tile(
                        [P, 2, BW], F8, tag=f"{wname}{o}", name=f"{wname}{o}_{blk}"
                    )
                    nc.sync.dma_start(wt[:], wv_[:, o, :, lo : lo + BW])
                    w8s[wname][o] = wt

                def load_wv(o):
                    wt = wblk.tile([P, BW], F16, tag=f"wv{o}", name=f"wv{o}_{blk}")
                    nc.sync.dma_start(wt[:], wv_v[:, o, lo : lo + BW])
                    wts["wv"][o] = wt

                if blk == 0:
                    # arrival order matches first consumption: the h=0 q-proj
                    # chain needs (xt8[o], wq[o]) pairs in o order
                    for o in range(KP):
                        nc.sync.dma_start(xt8s[o][:], xt8_v[:, o, :, :])
                        load_w8("wq", wq_v, o)
                    nc.sync.dma_start(bq_sb[:], bq_d[:])
                    nc.sync.dma_start(bk_sb[:], bk_d[:])
                    nc.sync.dma_start(bvb_sb[:], bvb_d[:])
                    nc.sync.dma_start(mask_sb[:], mask_d[:])
                    for o in range(KP):
                        load_w8("wk", wk_v, o)
                    # v path: fp16 x^T and wv interleaved in consumption order
                    for kc in range(KC):
                        nc.sync.dma_start(xts[kc][:], xt_v[:, kc, :])
                        load_wv(kc)
                else:
                    for o in range(KP):
                        load_w8("wq", wq_v, o)
                        load_w8("wk", wk_v, o)
                    for kc in range(KC):
                        load_wv(kc)

            wo_sb = res.tile([P, HL, C], F16, tag="wo")
            oT = res.tile([P, HL, T], F16, tag="oT")

            for blk in range(HL // HB):
                lo = blk * BW
                dma_block_weights(blk)

                qf = qkv.tile([P, HB, T], F16, tag="qf")
                kf = qkv.tile([P, HB, T], F16, tag="kf")
                vt = qkv.tile([P, T // P, BW], F16, tag="vt")

                # ---- Phase 1: projections for this block ----
                # q/k: fp8 DoubleRow, 8 chunk-pair matmuls per 512-col tile.
                # psum = SW * (x @ w); fused scale+bias moves it to SBUF fp16.
                for h in range(HB):
                    for dst, wname, bsb, sc in (
                        ("qf", "wq", bq_sb, rs / SW),
                        ("kf", "wk", bk_sb, 1.0 / SW),
                    ):
                        dtile = qf if dst == "qf" else kf
                        for t in range(T // 512):
                            pt = ps.tile([P, 512], F32, tag="mm")
                            for o in range(KP):
                                nc.tensor.matmul(
                                    pt[:],
                                    w8s[wname][o][:, :, h * DH : (h + 1) * DH],
                                    xt8s[o][:, :, t * 512 : (t + 1) * 512],
                                    start=(o == 0),
                                    stop=(o == KP - 1),
                                    perf_mode=DR,
                                )
                            nc.vector.scalar_tensor_tensor(
                                dtile[:, h, t * 512 : (t + 1) * 512],
                                pt[:],
                                sc,
                                bsb[
                                    :, blk * HB + h : blk * HB + h + 1
                                ].to_broadcast((P, 512)),
                                ALU.mult,
                                ALU.add,
                            )
                for m in range(T // P):
                    pt = ps.tile([P, 512], F32, tag="mm")
                    for kc in range(KC):
                        nc.tensor.matmul(
                            pt[:],
                            xts[kc][:, m * P : (m + 1) * P],
                            wts["wv"][kc][:],
                            start=(kc == 0),
                            stop=(kc == KC - 1),
                        )
                    nc.vector.tensor_tensor(
                        vt[:, m, :], pt[:], bvb_sb[:, lo : lo + BW], ALU.add
                    )

                if blk == 0:
                    # out-proj weights: needed only in phase 3; load mid-kernel
                    nc.sync.dma_start(
                        wo_sb[:], wo_d.rearrange("(h p) n -> p h n", p=P)
                    )

                # ---- Phase 2: causal attention, two heads interleaved ----
                for hp in range(HB // 2):
                    pair = (2 * hp, 2 * hp + 1)  # local head idx within block
                    for qc in range(T // 512):
                        jmax = (qc + 1) * 4
                        att = {}
                        den = {}
                        for l in pair:
                            att[l] = ps.tile(
                                [P, 512], F32, tag="att", bufs=3, name=f"att{l}"
                            )
                            den[l] = ps.tile(
                                [P, 512], F32, tag="den", bufs=2, name=f"den{l}"
                            )

                        def bounds(j):
                            s = max(512 * qc, 128 * j)
                            return s, 512 * qc + 512 - s

                        sts = {}

                        def issue_st(l, j):
                            s, n = bounds(j)
                            st = ps.tile([P, 512], F32, tag="mm", name=f"st{l}")
                            nc.tensor.matmul(
                                st[:, :n],
                                kf[:, l, j * P : (j + 1) * P],
                                qf[:, l, s : 512 * qc + 512],
                                start=True,
                                stop=True,
                            )
                            if 128 * j >= 512 * qc:
                                nc.vector.tensor_tensor(
                                    st[:, :P], st[:, :P], mask_sb[:], ALU.add
                                )
                            sts[(l, j)] = st

                        for l in pair:
                            issue_st(l, 0)
                        for j in range(jmax):
                            s, n = bounds(j)
                            c0 = s - 512 * qc
                            for l in pair:
                                st = sts.pop((l, j))
                                E = wp.tile([P, 512], F16, tag="E", bufs=6)
                                nc.scalar.activation(E[:, :n], st[:, :n], AF.Exp)
                                if j + 1 < jmax:
                                    issue_st(l, j + 1)
                                nc.tensor.matmul(
                                    att[l][:, c0:],
                                    vt[:, j, l * DH : (l + 1) * DH],
                                    E[:, :n],
                                    start=(j == 0),
                                    stop=(j == jmax - 1),
                                )
                                nc.tensor.matmul(
                                    den[l][:, c0:],
                                    ones_sb[:],
                                    E[:, :n],
                                    start=(j == 0),
                                    stop=(j == jmax - 1),
                                )
                        for l in pair:
                            hh = blk * HB + l
                            rc = wp.tile([P, 512], F32, tag="rc")
                            nc.vector.reciprocal_approx_fast(rc[:], den[l][:])
                            nc.vector.tensor_tensor(
                                oT[:, hh, qc * 512 : (qc + 1) * 512],
                                att[l][:],
                                rc[:],
                                ALU.mult,
                            )

            # ---- Phase 3: out projection (partial over this core's heads) ----
            part_v = part_d.rearrange("(mo p) n -> p mo n", p=P)
            for m in range(T // P):
                for n2 in range(C // 512):
                    pt = ps.tile([P, 512], F32, tag="mm")
                    for h in range(HL):
                        nc.tensor.matmul(
                            pt[:],
                            oT[:, h, m * P : (m + 1) * P],
                            wo_sb[:, h, n2 * 512 : (n2 + 1) * 512],
                            start=(h == 0),
                            stop=(h == HL - 1),
                        )
                    po = wp.tile([P, 512], F16, tag="po")
                    nc.vector.tensor_copy(po[:], pt[:])
                    nc.sync.dma_start(part_v[:, m, n2 * 512 : (n2 + 1) * 512], po[:])

    nc.compile()
    return nc


def _prep_inputs(x, w_qkv, b_qkv, w_out):
    """Build the 8 per-core input maps (host-side shard + layout prep)."""
    f16 = np.float16
    f8 = ml_dtypes.float8_e4m3
    rs = np.float32(1.0 / np.sqrt(DH))

    xt = [np.ascontiguousarray(x[b].T) for b in range(B)]
    xt16 = [a.astype(f16) for a in xt]
    xt8 = [a.astype(f8) for a in xt]

    mask = np.where(
        np.arange(P)[None, :] >= np.arange(P)[:, None], 0.0, -1e30
    ).astype(np.float32)

    per_g = []
    for g in range(2):
        lo, hi = g * HL * DH, (g + 1) * HL * DH
        wq = np.ascontiguousarray(w_qkv[:, lo:hi] * SW).astype(f8)
        wk = np.ascontiguousarray(w_qkv[:, C + lo : C + hi] * SW).astype(f8)
        wv = np.ascontiguousarray(w_qkv[:, 2 * C + lo : 2 * C + hi]).astype(f16)
        wo = np.ascontiguousarray(w_out[lo:hi, :]).astype(f16)
        bq = (b_qkv[lo:hi] * rs).astype(np.float32).reshape(HL, P).T.copy()
        bk = b_qkv[C + lo : C + hi].astype(np.float32).reshape(HL, P).T.copy()
        bv = b_qkv[2 * C + lo : 2 * C + hi].astype(np.float32)
        bvb = np.ascontiguousarray(np.broadcast_to(bv[None, :], (P, HL * DH)))
        per_g.append(dict(wq=wq, wk=wk, wv=wv, wo=wo, bq=bq, bk=bk, bvb=bvb))

    in_maps = []
    for c in range(NCORES):
        b, g = c // 2, c % 2
        m = dict(per_g[g])
        m["xt"] = xt16[b]
        m["xt8"] = xt8[b]
        m["mask"] = mask
        in_maps.append(m)
    return in_maps


def run(x, w_qkv, b_qkv, w_out, b_out, trace=False, **trace_kwargs):
    from concourse.bass_utils import run_bass_kernel_spmd

    x = np.asarray(x, dtype=np.float32)
    w_qkv = np.asarray(w_qkv, dtype=np.float32)
    b_qkv = np.asarray(b_qkv, dtype=np.float32)
    w_out = np.asarray(w_out, dtype=np.float32)
    b_out = np.asarray(b_out, dtype=np.float32)

    if "nc" not in _cache:
        _cache["nc"] = _build()
    nc = _cache["nc"]

    in_maps = _prep_inputs(x, w_qkv, b_qkv, w_out)
    res = run_bass_kernel_spmd(
        nc, in_maps, core_ids=list(range(NCORES)), trace=trace, **trace_kwargs
    )

    out = np.empty((B, T, C), np.float32)
    for b in range(B):
        out[b] = res.results[2 * b]["part"].astype(np.float32) + res.results[
            2 * b + 1
        ]["part"].astype(np.float32)
    out += b_out
    return out, res


def kernel(x, w_qkv, b_qkv, w_out, b_out):
    out, _ = run(x, w_qkv, b_qkv, w_out, b_out)
    return out


# revision 6
# speedup vs baseline: 1.2300x; 1.0078x over previous
"""Multi-head causal self-attention (B=4, T=1024, d_model=2048, 16 heads of 128)
for 8 Trainium2 NeuronCores.

Sharding: hybrid data x tensor parallel. Core c handles batch b = c//2 and
head group g = c%2 (8 heads per core). Each core computes q/k/v projections
for its 8 heads, causal flash-style attention, and the out-projection rows
for those heads, producing a partial [1024, 2048] output for its batch.
The host sums the two partials per batch and adds the output bias.

Precision: q/k projections run in fp8(e4m3) with DoubleRow matmuls (2 k-tiles
of 128 contracted per pass -> 2x PE throughput); softmax makes the resulting
~5% q/k noise nearly invisible in the output (rel err ~1e-2 vs the 2e-2 gate)
because attention-weight wiggle only re-mixes exact fp16 v rows. The v path,
scores, AV, denominator and out-projection stay fp16 (v and out-proj errors
pass straight through to the output, so fp8 there fails the gate). Weights
are pre-scaled by 64 before fp8 quantization (w ~ +-0.022 would be subnormal
in e4m3); the 1/64 and the 1/sqrt(dh) score scale are folded into the fused
scale+bias (scalar_tensor_tensor) that moves q/k from PSUM to SBUF.

All on-device layouts are feature-major so no transposes are needed anywhere:
  - x is shipped per batch as xt [2048, 1024] twice: fp16 (v path) and fp8
    in DoubleRow pair layout (q/k path)
  - q, k are produced feature-major [dh, T] per head; v token-major [T, dh]
  - scores are computed transposed: S^T[kv, q] = k_fm.T @ q_fm (lhsT=k, rhs=q)
  - softmax denominator via ones[128,128] matmul (partition reduction on PE),
    which also broadcasts the per-q sum to all 128 partitions
  - attention output accumulates as out^T[dh, q] = v_tm.T @ exp(S^T)
  - out^T is exactly the lhsT the out-projection needs

Heads are processed in two blocks of 4 so projection weights and q/k/v
activations fit in SBUF alongside the resident x^T and w_out. Within a
block, attention is computed for two heads interleaved so PE matmuls hide
the ACT exp latency. Inputs are DMA'd in per-k-chunk tiles so the first
projection matmuls start ~1us in instead of waiting for monolithic loads.
The output partial is written fp16 (halves the tail DMA); the host sums the
two partials per batch in fp32.
"""

import numpy as np
import ml_dtypes

B, T, C = 4, 1024, 2048
H = 16          # total heads
HL = 8          # heads per core (local)
HB = 4          # heads per block
DH = 128        # head dim
KC = C // 128   # contraction chunks (16)
KP = KC // 2    # DoubleRow chunk pairs (8)
P = 128
NCORES = 8
SW = 64.0       # fp8 weight pre-scale

_cache = {}


def _build():
    import concourse.bacc as bacc
    import concourse.mybir as mybir
    import concourse.tile as tile

    F32 = mybir.dt.float32
    F16 = mybir.dt.float16
    F8 = mybir.dt.float8e4
    AF = mybir.ActivationFunctionType
    ALU = mybir.AluOpType
    DR = mybir.MatmulPerfMode.DoubleRow

    rs = float(1.0 / np.sqrt(DH))

    nc = bacc.Bacc("TRN2", target_bir_lowering=False, debug=False)

    BW = HB * DH  # head-block feature width (512)

    # all inputs ship in partition-major pre-tiled layouts so every DMA is a
    # contiguous run on both the DRAM and SBUF side (max-size descriptors)
    xt_d = nc.dram_tensor("xt", (P, KC * T), F16, kind="ExternalInput")
    xt8_d = nc.dram_tensor("xt8", (P, KP * 2 * T), F8, kind="ExternalInput")
    wq_d = nc.dram_tensor("wq", (P, 2 * KP * 2 * BW), F8, kind="ExternalInput")
    wk_d = nc.dram_tensor("wk", (P, 2 * KP * 2 * BW), F8, kind="ExternalInput")
    wv_d = nc.dram_tensor("wv", (P, 2 * KC * BW), F16, kind="ExternalInput")
    wo_d = nc.dram_tensor("wo", (HL * DH, C), F16, kind="ExternalInput")
    bq_d = nc.dram_tensor("bq", (P, HL), F32, kind="ExternalInput")
    bk_d = nc.dram_tensor("bk", (P, HL), F32, kind="ExternalInput")
    bvb_d = nc.dram_tensor("bvb", (P, HL * DH), F32, kind="ExternalInput")
    mask_d = nc.dram_tensor("mask", (P, P), F32, kind="ExternalInput")
    part_d = nc.dram_tensor("part", (T, C), F16, kind="ExternalOutput")

    xt_v = xt_d.rearrange("p (o t) -> p o t", o=KC)
    xt8_v = xt8_d.rearrange("p (o two t) -> p o two t", o=KP, two=2)
    wq_v = wq_d.rearrange("p (b o two m) -> p b o two m", b=2, o=KP, two=2)
    wk_v = wk_d.rearrange("p (b o two m) -> p b o two m", b=2, o=KP, two=2)
    wv_v = wv_d.rearrange("p (b o m) -> p b o m", b=2, o=KC)

    with tile.TileContext(nc) as tc:
        with (
            tc.tile_pool(name="res", bufs=1) as res,
            tc.tile_pool(name="wblk", bufs=1) as wblk,
            tc.tile_pool(name="qkv", bufs=2) as qkv,
            tc.tile_pool(name="wp", bufs=3) as wp,
            tc.tile_pool(name="ps", bufs=3, space="PSUM") as ps,
        ):
            bq_sb = res.tile([P, HL], F32, tag="bq")
            bk_sb = res.tile([P, HL], F32, tag="bk")
            bvb_sb = res.tile([P, HL * DH], F32, tag="bvb")
            mask_sb = res.tile([P, P], F32, tag="mask")

            ones_sb = res.tile([P, P], F16, tag="ones")
            nc.vector.memset(ones_sb[:], 1.0)

            # Warm the PE (HAM un-throttles after ~3.4us of activity) while the
            # input DMAs stream in; these matmuls depend only on the memset.
            warm = ps.tile([P, P], F32, tag="mm")
            for _ in range(48):
                nc.tensor.matmul(warm[:], ones_sb[:], ones_sb[:], start=True, stop=True)

            # x^T fp16 per-k-chunk (v path) and fp8 per-pair (q/k path)
            xts = []
            for kc in range(KC):
                xts.append(res.tile([P, T], F16, tag=f"xt{kc}", name=f"xt{kc}"))
            xt8s = []
            for o in range(KP):
                xt8s.append(res.tile([P, 2, T], F8, tag=f"xt8_{o}", name=f"xt8_{o}"))
            wts = {"wv": [None] * KC}
            w8s = {"wq": [None] * KP, "wk": [None] * KP}

            def dma_block_weights(blk):
                def load_w8(wname, wv_, o):
                    wt = wblk.tile(
                        [P, 2, BW], F8, tag=f"{wname}{o}", name=f"{wname}{o}_{blk}"
                    )
                    nc.sync.dma_start(wt[:], wv_[:, blk, o, :, :])
                    w8s[wname][o] = wt

                def load_wv(o):
                    wt = wblk.tile([P, BW], F16, tag=f"wv{o}", name=f"wv{o}_{blk}")
                    nc.sync.dma_start(wt[:], wv_v[:, blk, o, :])
                    wts["wv"][o] = wt

                if blk == 0:
                    # arrival order matches first consumption: the h=0 q-proj
                    # chain needs (xt8[o], wq[o]) pairs in o order
                    for o in range(KP):
                        nc.sync.dma_start(xt8s[o][:], xt8_v[:, o, :, :])
                        load_w8("wq", wq_v, o)
                    nc.sync.dma_start(bq_sb[:], bq_d[:])
                    nc.sync.dma_start(bk_sb[:], bk_d[:])
                    nc.sync.dma_start(bvb_sb[:], bvb_d[:])
                    nc.sync.dma_start(mask_sb[:], mask_d[:])
                    for o in range(KP):
                        load_w8("wk", wk_v, o)
                    # v path: fp16 x^T and wv interleaved in consumption order
                    for kc in range(KC):
                        nc.sync.dma_start(xts[kc][:], xt_v[:, kc, :])
                        load_wv(kc)
                else:
                    for o in range(KP):
                        load_w8("wq", wq_v, o)
                        load_w8("wk", wk_v, o)
                    for kc in range(KC):
                        load_wv(kc)

            wo_sb = res.tile([P, HL, C], F16, tag="wo")
            oT = res.tile([P, HL, T], F16, tag="oT")

            for blk in range(HL // HB):
                lo = blk * BW
                dma_block_weights(blk)

                qf = qkv.tile([P, HB, T], F16, tag="qf")
                kf = qkv.tile([P, HB, T], F16, tag="kf")
                vt = qkv.tile([P, T // P, BW], F16, tag="vt")

                # ---- Phase 1: projections for this block ----
                # q/k: fp8 DoubleRow, 8 chunk-pair matmuls per 512-col tile.
                # psum = SW * (x @ w); fused scale+bias moves it to SBUF fp16.
                for h in range(HB):
                    for dst, wname, bsb, sc in (
                        ("qf", "wq", bq_sb, rs / SW),
                        ("kf", "wk", bk_sb, 1.0 / SW),
                    ):
                        dtile = qf if dst == "qf" else kf
                        for t in range(T // 512):
                            pt = ps.tile([P, 512], F32, tag="mm")
                            for o in range(KP):
                                nc.tensor.matmul(
                                    pt[:],
                                    w8s[wname][o][:, :, h * DH : (h + 1) * DH],
                                    xt8s[o][:, :, t * 512 : (t + 1) * 512],
                                    start=(o == 0),
                                    stop=(o == KP - 1),
                                    perf_mode=DR,
                                )
                            nc.vector.scalar_tensor_tensor(
                                dtile[:, h, t * 512 : (t + 1) * 512],
                                pt[:],
                                sc,
                                bsb[
                                    :, blk * HB + h : blk * HB + h + 1
                                ].to_broadcast((P, 512)),
                                ALU.mult,
                                ALU.add,
                            )
                for m in range(T // P):
                    pt = ps.tile([P, 512], F32, tag="mm")
                    for kc in range(KC):
                        nc.tensor.matmul(
                            pt[:],
                            xts[kc][:, m * P : (m + 1) * P],
                            wts["wv"][kc][:],
                            start=(kc == 0),
                            stop=(kc == KC - 1),
                        )
                    nc.vector.tensor_tensor(
                        vt[:, m, :], pt[:], bvb_sb[:, lo : lo + BW], ALU.add
                    )

                if blk == 0:
                    # out-proj weights: needed only in phase 3; load mid-kernel
                    nc.sync.dma_start(
                        wo_sb[:], wo_d.rearrange("(h p) n -> p h n", p=P)
                    )

                # ---- Phase 2: causal attention, two heads interleaved ----
                for hp in range(HB // 2):
                    pair = (2 * hp, 2 * hp + 1)  # local head idx within block
                    for qc in range(T // 512):
                        jmax = (qc + 1) * 4
                        att = {}
                        den = {}
                        for l in pair:
                            att[l] = ps.tile(
                                [P, 512], F32, tag="att", bufs=3, name=f"att{l}"
                            )
                            den[l] = ps.tile(
                                [P, 512], F32, tag="den", bufs=2, name=f"den{l}"
                            )

                        def bounds(j):
                            s = max(512 * qc, 128 * j)
                            return s, 512 * qc + 512 - s

                        sts = {}

                        def issue_st(l, j):
                            s, n = bounds(j)
                            st = ps.tile([P, 512], F32, tag="mm", name=f"st{l}")
                            nc.tensor.matmul(
                                st[:, :n],
                                kf[:, l, j * P : (j + 1) * P],
                                qf[:, l, s : 512 * qc + 512],
                                start=True,
                                stop=True,
                            )
                            if 128 * j >= 512 * qc:
                                nc.vector.tensor_tensor(
                                    st[:, :P], st[:, :P], mask_sb[:], ALU.add
                                )
                            sts[(l, j)] = st

                        for l in pair:
                            issue_st(l, 0)
                        for j in range(jmax):
                            s, n = bounds(j)
                            c0 = s - 512 * qc
                            for l in pair:
                                st = sts.pop((l, j))
                                E = wp.tile([P, 512], F16, tag="E", bufs=6)
                                nc.scalar.activation(E[:, :n], st[:, :n], AF.Exp)
                                if j + 1 < jmax:
                                    issue_st(l, j + 1)
                                nc.tensor.matmul(
                                    att[l][:, c0:],
                                    vt[:, j, l * DH : (l + 1) * DH],
                                    E[:, :n],
                                    start=(j == 0),
                                    stop=(j == jmax - 1),
                                )
                                nc.tensor.matmul(
                                    den[l][:, c0:],
                                    ones_sb[:],
                                    E[:, :n],
                                    start=(j == 0),
                                    stop=(j == jmax - 1),
                                )
                        for l in pair:
                            hh = blk * HB + l
                            rc = wp.tile([P, 512], F32, tag="rc")
                            nc.vector.reciprocal_approx_fast(rc[:], den[l][:])
                            nc.vector.tensor_tensor(
                                oT[:, hh, qc * 512 : (qc + 1) * 512],
                                att[l][:],
                                rc[:],
                                ALU.mult,
                            )

            # ---- Phase 3: out projection (partial over this core's heads) ----
            # stage full 2048-wide rows so each output DMA moves 4KB-contiguous
            # runs per partition instead of compute-paced 1KB slices
            part_v = part_d.rearrange("(mo p) n -> p mo n", p=P)
            for m in range(T // P):
                row = wp.tile([P, C], F16, tag="po")
                for n2 in range(C // 512):
                    pt = ps.tile([P, 512], F32, tag="mm")
                    for h in range(HL):
                        nc.tensor.matmul(
                            pt[:],
                            oT[:, h, m * P : (m + 1) * P],
                            wo_sb[:, h, n2 * 512 : (n2 + 1) * 512],
                            start=(h == 0),
                            stop=(h == HL - 1),
                        )
                    nc.vector.tensor_copy(row[:, n2 * 512 : (n2 + 1) * 512], pt[:])
                nc.sync.dma_start(part_v[:, m, :], row[:])

    nc.compile()
    return nc


def _prep_inputs(x, w_qkv, b_qkv, w_out):
    """Build the 8 per-core input maps (host-side shard + layout prep)."""
    f16 = np.float16
    f8 = ml_dtypes.float8_e4m3
    rs = np.float32(1.0 / np.sqrt(DH))

    BW = HB * DH

    def tile_x16(a):  # [C, T] -> [P, KC*T], chunk-major per partition
        return np.ascontiguousarray(
            a.reshape(KC, P, T).transpose(1, 0, 2).reshape(P, KC * T)
        )

    def tile_x8(a):  # [C, T] -> [P, KP*2*T], DoubleRow pair layout
        return np.ascontiguousarray(
            a.reshape(KP, 2, P, T).transpose(2, 0, 1, 3).reshape(P, KP * 2 * T)
        )

    def tile_w8(a):  # [C, 2*BW] -> [P, 2(blk)*KP*2*BW]
        return np.ascontiguousarray(
            a.reshape(KP, 2, P, 2, BW).transpose(2, 3, 0, 1, 4).reshape(P, -1)
        )

    def tile_wv(a):  # [C, 2*BW] -> [P, 2(blk)*KC*BW]
        return np.ascontiguousarray(
            a.reshape(KC, P, 2, BW).transpose(1, 2, 0, 3).reshape(P, -1)
        )

    xt = [x[b].T for b in range(B)]
    xt16 = [tile_x16(a.astype(f16)) for a in xt]
    xt8 = [tile_x8(a.astype(f8)) for a in xt]

    mask = np.where(
        np.arange(P)[None, :] >= np.arange(P)[:, None], 0.0, -1e30
    ).astype(np.float32)

    per_g = []
    for g in range(2):
        lo, hi = g * HL * DH, (g + 1) * HL * DH
        wq = tile_w8((w_qkv[:, lo:hi] * SW).astype(f8))
        wk = tile_w8((w_qkv[:, C + lo : C + hi] * SW).astype(f8))
        wv = tile_wv(w_qkv[:, 2 * C + lo : 2 * C + hi].astype(f16))
        wo = np.ascontiguousarray(w_out[lo:hi, :]).astype(f16)
        bq = (b_qkv[lo:hi] * rs).astype(np.float32).reshape(HL, P).T.copy()
        bk = b_qkv[C + lo : C + hi].astype(np.float32).reshape(HL, P).T.copy()
        bv = b_qkv[2 * C + lo : 2 * C + hi].astype(np.float32)
        bvb = np.ascontiguousarray(np.broadcast_to(bv[None, :], (P, HL * DH)))
        per_g.append(dict(wq=wq, wk=wk, wv=wv, wo=wo, bq=bq, bk=bk, bvb=bvb))

    in_maps = []
    for c in range(NCORES):
        b, g = c // 2, c % 2
        m = dict(per_g[g])
        m["xt"] = xt16[b]
        m["xt8"] = xt8[b]
        m["mask"] = mask
        in_maps.append(m)
    return in_maps


def run(x, w_qkv, b_qkv, w_out, b_out, trace=False, **trace_kwargs):
    from concourse.bass_utils import run_bass_kernel_spmd

    x = np.asarray(x, dtype=np.float32)
    w_qkv = np.asarray(w_qkv, dtype=np.float32)
    b_qkv = np.asarray(b_qkv, dtype=np.float32)
    w_out = np.asarray(w_out, dtype=np.float32)
    b_out = np.asarray(b_out, dtype=np.float32)

    if "nc" not in _cache:
        _cache["nc"] = _build()
    nc = _cache["nc"]

    in_maps = _prep_inputs(x, w_qkv, b_qkv, w_out)
    res = run_bass_kernel_spmd(
        nc, in_maps, core_ids=list(range(NCORES)), trace=trace, **trace_kwargs
    )

    out = np.empty((B, T, C), np.float32)
    for b in range(B):
        out[b] = res.results[2 * b]["part"].astype(np.float32) + res.results[
            2 * b + 1
        ]["part"].astype(np.float32)
    out += b_out
    return out, res


def kernel(x, w_qkv, b_qkv, w_out, b_out):
    out, _ = run(x, w_qkv, b_qkv, w_out, b_out)
    return out
